# revision 7
# baseline (speedup 1.0000x reference)
"""Causal single-head attention block for Trainium2, SPMD across 8 NeuronCores.

Problem (hardcoded):
    x:     [4, 2048, 1024] f32
    w_qkv: [1024, 3072]    f32   (q | k | v column blocks)
    w_out: [1024, 1024]    f32
    b_out: [1024]          f32
    y = softmax(causal(q @ k.T / 32)) @ v @ w_out + b_out     -> [4, 2048, 1024]

Algebraic folding (host-side, fp32):
    sim  = (x wq)(x wk)^T = x (wq wk^T) x^T          -> Mq  = wq @ wk.T
    out  = attn (x wv) wo = attn x (wv wo)           -> Wvo = wv @ w_out
so the device kernel never materializes Q/K/V: it computes
    QM^T = Mq^T x_q^T   (local queries only)
    sim  = QM x^T       (x^T SBUF-resident)
    attnX = softmax(causal(sim)) @ x                 (x rows streamed)
    y    = attnX @ Wvo + b
This halves the tensor-engine work vs. projecting Q/K/V explicitly and
removes the duplicated K/V computation across the core pair.

Sharding: 2 cores per batch element. Within a batch, the 16 query subtiles of
128 rows are dealt round-robin to the core pair (core parity h gets subtiles
s = 2k + h, k = 0..7) so both cores see the identical causal work profile
(512-key chunk counts [1,1,2,2,3,3,4,4]) and a single SPMD program serves all
8 cores; per-core behavior differs only through input data.

The group loop is software-pipelined: group g+1's sim matmuls are issued
ahead of group g's PV/projection so the tensor engine fills group g's
softmax (mask->exp->normalize) latency instead of idling, and stays HAM-warm.

All matmul operands are bf16 (PSUM accumulation in fp32; softmax statistics
in fp32): the elementwise rounding step is 4x fp32r's, far inside the
tolerance, and bf16 enables fast weight load + halves DMA/DVE traffic.
"""

import numpy as np

import concourse.mybir as mybir
import concourse.tile as tile
from concourse import bacc
from concourse.bass_utils import run_bass_kernel_spmd

FP32 = mybir.dt.float32
BF16 = mybir.dt.bfloat16
AF = mybir.ActivationFunctionType
ALU = mybir.AluOpType

B, S, D, NI, NO = 4, 2048, 1024, 1024, 1024
NCORES = 8
P = 128
DC = D // P    # 8 contraction chunks for the projections
IC = NI // P   # 8 inner-dim chunks
RC = S // 512  # 4 key chunks
NSUB = 8       # local 128-row query subtiles per core
CC = [k // 2 + 1 for k in range(NSUB)]  # 512-key chunks per local subtile
SCALE = float(NI) ** -0.5
NEG = -1.0e9

_CACHED = {}


def _build():
    nc = bacc.Bacc(None, target_bir_lowering=False, debug=False, num_devices=NCORES)

    xT = nc.dram_tensor("xT", [D, S], BF16, kind="ExternalInput").ap()
    xQ = nc.dram_tensor("xQ", [D, NSUB * P], BF16, kind="ExternalInput").ap()
    xR = nc.dram_tensor("xR", [S, D], BF16, kind="ExternalInput").ap()
    mq_d = nc.dram_tensor("mq", [D, D], BF16, kind="ExternalInput").ap()
    wvo_d = nc.dram_tensor("wvo", [NI, NO], BF16, kind="ExternalInput").ap()
    masks = nc.dram_tensor("masks", [NSUB, P, 512], BF16, kind="ExternalInput").ap()
    bb = nc.dram_tensor("bb", [P, NO], FP32, kind="ExternalInput").ap()
    ident = nc.dram_tensor("ident", [P, P], BF16, kind="ExternalInput").ap()
    y = nc.dram_tensor("y", [NSUB * P, NO], BF16, kind="ExternalOutput").ap()

    with tile.TileContext(nc) as tc:
        with (
            tc.tile_pool(name="const", bufs=1) as constp,
            tc.tile_pool(name="xtpool", bufs=IC) as xtp,
            tc.tile_pool(name="qtpool0", bufs=IC) as qtp0,
            tc.tile_pool(name="qtpool1", bufs=IC) as qtp1,
        ):
            XT = [xtp.tile([P, S], BF16, name=f"xt{i}", tag="xt") for i in range(IC)]
            # per-qh-half QM^T tiles: keeps group 0's sim free of any
            # dependency on the second half's PSUM->SBUF copies
            QMT = [
                [qtp0.tile([P, 512], BF16, name=f"qt0_{i}", tag="qt0")
                 for i in range(IC)],
                [qtp1.tile([P, 512], BF16, name=f"qt1_{i}", tag="qt1")
                 for i in range(IC)],
            ]

            # ---- Phase 0: QM^T = Mq^T @ xQ for all 1024 local queries ----
            # d-outer accumulation into 8 concurrently-open PSUM banks: the
            # first matmul only needs mq[0] cols + xq[0] on chip, so the PE
            # starts as soon as the first two transfers land.
            with tc.tile_pool(name="qacc", bufs=IC, space="PSUM") as qacc, \
                 tc.tile_pool(name="wpool", bufs=DC) as wp, \
                 tc.tile_pool(name="xqp", bufs=2 * DC) as xqp:
                mq = []
                xqs = [[], []]
                for d in range(DC):
                    wt = wp.tile([P, D], BF16, name=f"mq{d}", tag="w")
                    if d == 0:
                        nc.sync.dma_start(out=wt[:, 0:512],
                                          in_=mq_d[0:P, 0:512])
                    t = xqp.tile([P, 512], BF16, name=f"xq0_{d}", tag="xq")
                    nc.sync.dma_start(out=t[:], in_=xQ[P * d:P * (d + 1), 0:512])
                    xqs[0].append(t)
                    if d == 0:
                        nc.sync.dma_start(out=wt[:, 512:1024],
                                          in_=mq_d[0:P, 512:1024])
                    else:
                        nc.sync.dma_start(out=wt[:],
                                          in_=mq_d[P * d:P * (d + 1), :])
                    mq.append(wt)
                for d in range(DC):
                    t = xqp.tile([P, 512], BF16, name=f"xq1_{d}", tag="xq")
                    nc.sync.dma_start(out=t[:], in_=xQ[P * d:P * (d + 1), 512:1024])
                    xqs[1].append(t)
                # x^T tiles (whole rows, one descriptor each) land well before
                # group 0's sim consumes chunk 0
                for i in range(IC):
                    nc.sync.dma_start(out=XT[i][:], in_=xT[P * i:P * (i + 1), :])
                mask_sb = constp.tile([P, NSUB, 512], BF16, name="mask_sb", tag="mask")
                for k in range(NSUB):
                    nc.sync.dma_start(out=mask_sb[:, k, :], in_=masks[k])
                ident_sb = constp.tile([P, P], BF16, name="ident_sb", tag="id")
                nc.sync.dma_start(out=ident_sb[:], in_=ident[:])

                for qh in range(2):
                    pss = [
                        qacc.tile([P, 512], FP32, name=f"ps_qt{qh}_{i}", tag="qa")
                        for i in range(IC)
                    ]
                    for d in range(DC):
                        for i in range(IC):
                            nc.tensor.matmul(
                                pss[i][:], mq[d][:, P * i:P * (i + 1)],
                                xqs[qh][d][:],
                                start=(d == 0), stop=(d == DC - 1),
                            )
                    # qh=0 copies on DVE, qh=1 on the scalar engine: the two
                    # halves drain in parallel instead of queueing on one
                    # engine (qh=1 isn't consumed until group 2, so the
                    # slower ACT copy is off the critical path)
                    for i in range(IC):
                        if qh == 0:
                            nc.vector.tensor_copy(QMT[qh][i][:], pss[i][:])
                        else:
                            nc.scalar.copy(QMT[qh][i][:], pss[i][:])

            # ---- attention, 4 pair-groups of 2 subtiles, pipelined ----
            with (
                tc.tile_pool(name="accp", bufs=2, space="PSUM") as accp,
                tc.tile_pool(name="tpp", bufs=2, space="PSUM") as tpp,
                tc.tile_pool(name="opp", bufs=4, space="PSUM") as opp,
                tc.tile_pool(name="wopool", bufs=DC) as wop,
                tc.tile_pool(name="vfixp", bufs=4) as vfixp,
            ):
                # x rows [0:512) are read by every group: pin them in SBUF
                vfix = []
                for t in range(4):
                    vf = vfixp.tile([P, NI], BF16, name=f"vfix{t}", tag="vfix")
                    nc.sync.dma_start(out=vf[:], in_=xR[P * t:P * (t + 1), :])
                    vfix.append(vf)
                wo = []
                for d in range(DC):
                    t = wop.tile([P, NO], BF16, name=f"wo{d}", tag="wo")
                    nc.sync.dma_start(out=t[:], in_=wvo_d[P * d:P * (d + 1), :])
                    wo.append(t)
                b_sb = constp.tile([P, NO], FP32, name="b_sb", tag="b")
                nc.sync.dma_start(out=b_sb[:], in_=bb[:])
                with (
                    tc.tile_pool(name="ppool", bufs=4) as ppool,
                    tc.tile_pool(name="ptpool", bufs=3) as ptpool,
                    tc.tile_pool(name="otpool", bufs=2 * IC) as otpool,
                    tc.tile_pool(name="vrd", bufs=4) as vrdp,
                    tc.tile_pool(name="ypool", bufs=4) as ypool,
                    tc.tile_pool(name="stp", bufs=12) as stp,
                ):
                    PS = {}

                    def sim_phase(g):
                        L = g + 1
                        for k in (2 * g, 2 * g + 1):
                            p_t = ppool.tile([P, 4 * 512], BF16, name=f"p{k}",
                                             tag="p")
                            sums = stp.tile([P, 4], FP32, name=f"sums{k}",
                                            tag="sums")
                            # diagonal chunk first: its mask+exp chain overlaps
                            # the remaining chunks' matmuls
                            for kc in ([L - 1] + list(range(L - 1))):
                                ps = accp.tile([P, 512], FP32, name="ps_sim",
                                               tag="acc")
                                for i in range(IC):
                                    nc.tensor.matmul(
                                        ps[:],
                                        QMT[k // 4][i][:, P * (k % 4):
                                                       P * (k % 4) + P],
                                        XT[i][:, 512 * kc:512 * (kc + 1)],
                                        start=(i == 0), stop=(i == IC - 1),
                                    )
                                if kc == L - 1:
                                    nc.vector.tensor_tensor(
                                        out=ps[:], in0=ps[:], in1=mask_sb[:, k, :],
                                        op=ALU.add,
                                    )
                                nc.scalar.activation(
                                    p_t[:, 512 * kc:512 * (kc + 1)], ps[:], AF.Exp,
                                    scale=SCALE, accum_out=sums[:, kc:kc + 1],
                                )
                            ssum = stp.tile([P, 1], FP32, name=f"ssum{k}", tag="ss")
                            nc.vector.tensor_reduce(
                                ssum[:], sums[:, :L], axis=mybir.AxisListType.X,
                                op=ALU.add,
                            )
                            rsum = stp.tile([P, 1], FP32, name=f"rsum{k}", tag="rs")
                            nc.vector.reciprocal(rsum[:], ssum[:])
                            nc.vector.tensor_scalar_mul(
                                p_t[:, :512 * L], p_t[:, :512 * L], rsum[:]
                            )
                            PS[k] = p_t

                    def pv_proj(g):
                        L = g + 1
                        k0, k1 = 2 * g, 2 * g + 1
                        ops = [
                            opp.tile([P, 512], FP32, name=f"op{g}_{j}", tag="op")
                            for j in range(4)
                        ]
                        nt = 4 * L
                        for t in range(nt):
                            tp_ps = tpp.tile([P, 256], BF16, name="tp", tag="tp")
                            nc.tensor.transpose(
                                tp_ps[:, 0:P], PS[k0][:, P * t:P * (t + 1)],
                                ident_sb[:]
                            )
                            nc.tensor.transpose(
                                tp_ps[:, P:256], PS[k1][:, P * t:P * (t + 1)],
                                ident_sb[:]
                            )
                            pt_t = ptpool.tile([P, 256], BF16, name="pt", tag="pt")
                            nc.vector.tensor_copy(pt_t[:], tp_ps[:])
                            if t < 4:
                                v_t = vfix[t]
                            else:
                                v_t = vrdp.tile([P, NI], BF16, name="v_t", tag="v")
                                nc.sync.dma_start(
                                    out=v_t[:], in_=xR[P * t:P * (t + 1), :]
                                )
                            for m in range(IC):
                                # one accumulation group per PSUM bank: start
                                # only on the bank's first matmul (whole-bank
                                # pending-zero makes the sibling column-half's
                                # first write an overwrite), stop on its last
                                nc.tensor.matmul(
                                    ops[m // 2][:, 256 * (m % 2):
                                                256 * (m % 2) + 256],
                                    v_t[:, P * m:P * (m + 1)],
                                    pt_t[:],
                                    start=(t == 0 and m % 2 == 0),
                                    stop=(t == nt - 1 and m % 2 == 1),
                                )

                        oT = []
                        for m in range(IC):
                            ot = otpool.tile([P, 256], BF16, name=f"ot{g}_{m}",
                                             tag="ot")
                            nc.vector.tensor_copy(
                                ot[:],
                                ops[m // 2][:, 256 * (m % 2):256 * (m % 2) + 256]
                            )
                            oT.append(ot)

                        # ---- output projection for this group's 2 subtiles ----
                        # y psums cycle through the opp pool so accp stays free
                        # for the pipelined sim matmuls; bias-add runs on the
                        # otherwise-idle GpSimd engine
                        for col, k in enumerate((k0, k1)):
                            for oh in range(2):
                                ps = opp.tile([P, 512], FP32, name="ps_y", tag="op")
                                for i in range(IC):
                                    nc.tensor.matmul(
                                        ps[:],
                                        oT[i][:, P * col:P * (col + 1)],
                                        wo[i][:, 512 * oh:512 * (oh + 1)],
                                        start=(i == 0), stop=(i == IC - 1),
                                    )
                                y_sb = ypool.tile([P, 512], BF16, name="y_sb",
                                                  tag="y")
                                nc.vector.tensor_tensor(
                                    out=y_sb[:], in0=ps[:],
                                    in1=b_sb[:, 512 * oh:512 * (oh + 1)],
                                    op=ALU.add,
                                )
                                nc.sync.dma_start(
                                    out=y[P * k:P * (k + 1),
                                          512 * oh:512 * (oh + 1)],
                                    in_=y_sb[:],
                                )

                    sim_phase(0)
                    for g in range(4):
                        if g < 3:
                            sim_phase(g + 1)
                        pv_proj(g)

    nc.compile()
    return nc


def _prep_inputs(x, w_qkv, w_out, b_out):
    import ml_dtypes
    BF = ml_dtypes.bfloat16
    x = np.asarray(x, dtype=np.float32)
    w_qkv = np.asarray(w_qkv, dtype=np.float32)
    w_out = np.asarray(w_out, dtype=np.float32)
    b_out = np.asarray(b_out, dtype=np.float32)

    wq = w_qkv[:, 0 * NI:1 * NI]
    wk = w_qkv[:, 1 * NI:2 * NI]
    wv = w_qkv[:, 2 * NI:3 * NI]
    mq = np.ascontiguousarray((wq @ wk.T).astype(BF))
    wvo = np.ascontiguousarray((wv @ w_out).astype(BF))
    b_bcast = np.ascontiguousarray(np.broadcast_to(b_out[None, :], (P, NO)))
    ident = np.eye(P, dtype=BF)

    xbf = [x[b].astype(BF) for b in range(B)]
    xTs = [np.ascontiguousarray(xb.T) for xb in xbf]

    in_maps = []
    for c in range(NCORES):
        b, h = c // 2, c % 2
        subs = [2 * k + h for k in range(NSUB)]
        xQc = np.concatenate(
            [xTs[b][:, P * s:P * (s + 1)] for s in subs], axis=1
        )
        m = np.empty((NSUB, P, 512), dtype=BF)
        cpos = np.arange(512)[None, :]
        prow = np.arange(P)[:, None]
        for k in range(NSUB):
            off = P * subs[k] - 512 * (CC[k] - 1)
            m[k] = np.where(cpos <= off + prow, 0.0, NEG)
        in_maps.append({
            "xT": xTs[b], "xQ": np.ascontiguousarray(xQc), "xR": xbf[b],
            "mq": mq, "wvo": wvo,
            "masks": m, "bb": b_bcast, "ident": ident,
        })
    return in_maps


def _run(x, w_qkv, w_out, b_out, trace=False, **kw):
    if "nc" not in _CACHED:
        _CACHED["nc"] = _build()
    nc = _CACHED["nc"]
    in_maps = _prep_inputs(x, w_qkv, w_out, b_out)
    res = run_bass_kernel_spmd(nc, in_maps, list(range(NCORES)), trace=trace, **kw)
    out = np.empty((B, S, NO), dtype=np.float32)
    for c in range(NCORES):
        b, h = c // 2, c % 2
        yc = np.asarray(res.results[c]["y"], dtype=np.float32)
        for k in range(NSUB):
            s = 2 * k + h
            out[b, P * s:P * (s + 1), :] = yc[P * k:P * (k + 1), :]
    return out, res


def kernel(x, w_qkv, w_out, b_out):
    out, _ = _run(x, w_qkv, w_out, b_out, trace=False)
    return out
